# revision 24
# baseline (speedup 1.0000x reference)
"""MoBA (Mixture of Block Attention) forward — Trainium2 Bass kernel.

Problem shapes: B=2, S=4096, DIM=1024, H=16 heads, HD=64, BLK=512, TOPK=3.

Sharding: 8 cores = 2 batches x 4 head-groups (4 heads each).
Each core computes, for its (batch b, heads hg*4..hg*4+3):
  q/k/v projections (head-sliced weights), RoPE, block gating + top-3,
  gathered block attention, and a partial output projection
  (row-slice of wo).  The host sums the 4 partials per batch.

Per-core layouts (all fp32):
  xT   [1024, 4096]  x[b] transposed (host-prepped)
  wq   [1024, 256]   head-sliced, pre-scaled by 1/sqrt(HD)
  wk   [1024, 256]
  wv   [1024, 256]
  wo   [256, 1024]   row-sliced
  cc   [128, 4096]   cos.T tiled 4x vertically
  ss   [128, 4096]   [-sin.T; +sin.T] tiled 2x
  mask [8, 8]        (c, blk): 0 if blk<=c else -(1e6+1000*blk)
  out  [4096, 1024]  partial output

On-device dataflow (per core):
  Phase A: for each s-tile of 512: Q^T,K^T ([dout,s] layout, 2 tiles of
    128 rows = 2 heads each) + RoPE (partition-swap via DMA) + per-block
    K sums; V in natural layout packed per (head, s-chunk-of-128) with an
    appended ones column (for softmax sums via the PV matmul).
  Phase C: per chunk c: gating scores at the chunk-mid query against
    block K-sums (tiny matmuls), +mask, top-3 via DVE max8/max_index;
    per head: gather the 3 selected K^T/V blocks via dynamic-offset
    SBUF->SBUF DMA, then 12x [QK^T -> exp -> PV] accumulating into a
    [65, 512] PSUM whose last row is sum(exp); normalize via reciprocal
    + K=1 broadcast matmul; o-projection of the finished chunk.
"""

import numpy as np

import concourse.bass as bass
import concourse.bacc as bacc
import concourse.mybir as mybir
from concourse.tile import TileContext
from concourse import bass_utils

F32 = mybir.dt.float32
F32R = mybir.dt.float32r
F16 = mybir.dt.float16
U32 = mybir.dt.uint32
EXP = mybir.ActivationFunctionType.Exp
COPY = mybir.ActivationFunctionType.Copy
AX_X = mybir.AxisListType.X
SP = mybir.EngineType.SP

S = 4096
DIN = 1024
HD = 64
NH = 4          # heads per core
BLK = 512
NB = 8          # blocks
NST = 8         # s-tiles of 512
TOPK = 3
VSTRIDE = 65    # V chunk cols (64 + ones col)
VBLK = 4 * VSTRIDE      # 260: one block of V per head
VHEAD = NB * VBLK       # 2080: all blocks of V per head


def _r(ap):
    return ap


def _build_body(nc, tc, t, ctx):
    ds = bass.ds
    xT, wq, wk, wv, wo, cc, ss, mask, out = (
        t["xT"], t["wq"], t["wk"], t["wv"], t["wo"], t["cc"], t["ss"],
        t["mask"], t["out"])
    wqs, wks = t["wqs"], t["wks"]

    res = ctx.enter_context(tc.tile_pool(name="resident", bufs=1))

    qT = [res.tile([128, S], F16, tag=f"qT{dt}", name=f"qT{dt}") for dt in range(2)]
    kT = [res.tile([128, S], F16, tag=f"kT{dt}", name=f"kT{dt}") for dt in range(2)]
    vsb = res.tile([128, NH * VHEAD], F16, tag="vsb")
    wo_sb = res.tile([128, 2, 1024], F16, tag="wo")
    bsum = [res.tile([128, NB], F32, tag=f"bsum{dt}", name=f"bsum{dt}") for dt in range(2)]
    qmid = [res.tile([128, NB], F32, tag=f"qmid{dt}", name=f"qmid{dt}") for dt in range(2)]
    mask_sb = res.tile([8, 8], F32, tag="mask")
    ones_sb = res.tile([1, 64], F16, tag="ones")

    nc.sync.dma_start(wo_sb, wo.rearrange("(c p) d -> p c d", p=128))
    nc.sync.dma_start(mask_sb, mask[:, :])
    # memset can't write f32r; memset fp32 scratch then DVE-copy (rounds)
    ones_f32 = res.tile([128, 128], F32, tag="onesf32")
    nc.vector.memset(ones_f32, 1.0)
    nc.vector.tensor_copy(ones_sb, ones_f32[0:1, 0:64])
    # ones columns of vsb: free offsets h*VHEAD + sub*VSTRIDE + 64
    vones = vsb[:].rearrange("p (h s e) -> p h s e", h=NH, s=NB * 4)[:, :, :, 64:65]
    nc.vector.tensor_copy(
        vones, ones_f32[:].rearrange("p (a b c) -> p a b c", a=NH, b=NB * 4))

    gidx = [res.tile([8, 8], U32, tag=f"gidx{h}", name=f"gidx{h}")
            for h in range(NH)]
    koff = [res.tile([8, 8], U32, tag=f"koff{h}", name=f"koff{h}")
            for h in range(NH)]
    voff = [res.tile([8, 8], U32, tag=f"voff{h}", name=f"voff{h}")
            for h in range(NH)]
    koff_w = [None] * NH
    voff_w = [None] * NH

    # ---------------- Phase A: projections + RoPE + block sums ----------
    with (
        tc.tile_pool(name="wqkv", bufs=1) as wpool,
        tc.tile_pool(name="xt", bufs=16) as xpool,
        tc.tile_pool(name="ccss", bufs=4) as cpool,
        tc.tile_pool(name="scr", bufs=10) as spool,
        tc.tile_pool(name="prjp", bufs=6, space="PSUM") as ppool,
    ):
        wq_sb = wpool.tile([128, 8, 256], F16, tag="wq")
        wk_sb = wpool.tile([128, 8, 256], F16, tag="wk")
        wqs_sb = wpool.tile([128, 8, 256], F16, tag="wqs")
        wks_sb = wpool.tile([128, 8, 256], F16, tag="wks")
        wv_sb = wpool.tile([128, 8, 256], F16, tag="wv")
        nc.sync.dma_start(wq_sb, wq.rearrange("(c p) d -> p c d", p=128))
        nc.sync.dma_start(wk_sb, wk.rearrange("(c p) d -> p c d", p=128))
        nc.sync.dma_start(wqs_sb, wqs.rearrange("(c p) d -> p c d", p=128))
        nc.sync.dma_start(wks_sb, wks.rearrange("(c p) d -> p c d", p=128))
        nc.sync.dma_start(wv_sb, wv.rearrange("(c p) d -> p c d", p=128))

        for st in range(NST):
            sl = slice(st * 512, (st + 1) * 512)
            xt = []
            for di in range(8):
                x_tile = xpool.tile([128, 512], F16, tag="xt")
                nc.sync.dma_start(x_tile, xT[di * 128:(di + 1) * 128, sl])
                xt.append(x_tile)
            cc_sl = cpool.tile([128, 512], F32, tag="ccss")
            ss_sl = cpool.tile([128, 512], F32, tag="ccss")
            nc.sync.dma_start(cc_sl, cc[:, sl])
            nc.sync.dma_start(ss_sl, ss[:, sl])

            for w_sb, w_swp, dst, is_k in ((wq_sb, wqs_sb, qT, False),
                                           (wk_sb, wks_sb, kT, True)):
                for dt in range(2):
                    psum = ppool.tile([128, 512], F32, tag="prj")
                    psws = ppool.tile([128, 512], F32, tag="prj")
                    for di in range(8):
                        nc.tensor.matmul(
                            psum,
                            lhsT=_r(w_sb[:, di, dt * 128:(dt + 1) * 128]),
                            rhs=_r(xt[di]),
                            start=(di == 0), stop=(di == 7))
                    for di in range(8):
                        nc.tensor.matmul(
                            psws,
                            lhsT=_r(w_swp[:, di, dt * 128:(dt + 1) * 128]),
                            rhs=_r(xt[di]),
                            start=(di == 0), stop=(di == 7))
                    # RoPE (swap pre-folded into w_swp): out = p*cc + ps*ss
                    t1 = spool.tile([128, 512], F32, tag="scr")
                    t2 = spool.tile([128, 512], F32, tag="scr")
                    full = spool.tile([128, 512], F32, tag="scr")
                    nc.vector.tensor_mul(t1, psum, cc_sl)
                    nc.vector.tensor_mul(t2, psws, ss_sl)
                    nc.vector.tensor_add(full, t1, t2)
                    nc.vector.tensor_copy(dst[dt][:, sl], full)
                    if is_k:
                        nc.vector.reduce_sum(bsum[dt][:, st:st + 1],
                                             full, axis=AX_X)
                    else:
                        nc.vector.tensor_copy(qmid[dt][:, st:st + 1],
                                              full[:, 256:257])

            for sub in range(4):
                vpsum = ppool.tile([128, 256], F32, tag="prj")
                for di in range(8):
                    nc.tensor.matmul(
                        vpsum,
                        lhsT=_r(xt[di][:, sub * 128:(sub + 1) * 128]),
                        rhs=_r(wv_sb[:, di, :]),
                        start=(di == 0), stop=(di == 7))
                gsub = st * 4 + sub
                vdst = vsb[:].rearrange(
                    "p (h s e) -> p h s e", h=NH, s=NB * 4)[:, :, gsub, 0:64]
                vsrc = vpsum[:].rearrange("p (h d) -> p h d", h=NH)
                nc.vector.tensor_copy(vdst, vsrc)

        # ------------ gating + top-3 (uses prjp psum pool) --------------
        for h in range(NH):
            dt, r0 = h // 2, (h % 2) * 64
            gpsum = ppool.tile([8, 8], F32, tag="prj", name=f"gp{h}")
            nc.tensor.matmul(gpsum, lhsT=qmid[dt][r0:r0 + 64, :],
                             rhs=bsum[dt][r0:r0 + 64, :],
                             start=True, stop=True)
            gat = res.tile([8, 8], F32, tag=f"gat{h}", name=f"gat{h}")
            nc.vector.tensor_add(gat, gpsum, mask_sb)
            gmax = res.tile([8, 8], F32, tag=f"gmax{h}", name=f"gmax{h}")
            nc.vector.max(gmax, gat)
            nc.vector.max_index(gidx[h], gmax, gat)
            koff_w[h] = nc.vector.tensor_scalar(koff[h], gidx[h], 512, None,
                                    op0=mybir.AluOpType.mult)
            voff_w[h] = nc.vector.tensor_scalar(voff[h], gidx[h], VBLK, h * VHEAD,
                                    op0=mybir.AluOpType.mult,
                                    op1=mybir.AluOpType.add)

    from concourse.tile_rust import add_dep_helper

    # ---------------- Phase C: attention + o-proj -----------------------
    prev_gathers = []
    with (
        tc.tile_pool(name="sel", bufs=9) as selpool,
        tc.tile_pool(name="esb", bufs=8) as epool,
        tc.tile_pool(name="att", bufs=4) as apool,
        tc.tile_pool(name="oo", bufs=2) as opool,
        tc.tile_pool(name="rcp", bufs=2) as rpool,
        tc.tile_pool(name="scp", bufs=4, space="PSUM") as scpool,
        tc.tile_pool(name="avp", bufs=4, space="PSUM") as avpool,
    ):
        oppool = scpool
        def emit_gathers(c, h):
            kld, ksels = nc.values_load_multi_w_load_instructions(
                koff[h][c:c + 1, 0:TOPK], engines=(SP,),
                min_val=0, max_val=7 * 512, skip_runtime_bounds_check=True)
            vld, vsels = nc.values_load_multi_w_load_instructions(
                voff[h][c:c + 1, 0:TOPK], engines=(SP,),
                min_val=h * VHEAD, max_val=h * VHEAD + 7 * VBLK,
                skip_runtime_bounds_check=True)
            for ld in kld:
                add_dep_helper(ld.ins, koff_w[h].ins, sync=True,
                               reason="reg_load RAW on koff")
            for ld in vld:
                add_dep_helper(ld.ins, voff_w[h].ins, sync=True,
                               reason="reg_load RAW on voff")
            for ld in (*kld, *vld):
                for g in prev_gathers:
                    add_dep_helper(ld.ins, g.ins, sync=False,
                                   reason="reg reuse WAR")
            ksel = selpool.tile([128, TOPK * 512], F16, tag="ksel",
                                name=f"ksel{c}_{h}")
            vsel = selpool.tile([128, TOPK * VBLK], F16, tag="vsel",
                                name=f"vsel{c}_{h}")
            prev_gathers.clear()
            dt = h // 2
            for sl_i in range(TOPK):
                prev_gathers.append(nc.sync.dma_start(
                    ksel[:, sl_i * 512:(sl_i + 1) * 512],
                    kT[dt][:, ds(ksels[sl_i], 512)]))
                prev_gathers.append(nc.sync.dma_start(
                    vsel[:, sl_i * VBLK:(sl_i + 1) * VBLK],
                    vsb[:, ds(vsels[sl_i], VBLK)]))
            return ksel, vsel

        def attn_head_ctx(c, h, ksel, vsel):
            dt, r0 = h // 2, (h % 2) * 64
            csl = slice(c * 512, (c + 1) * 512)
            apsum = avpool.tile([65, 512], F32, tag="av", name=f"ap{c}_{h}")

            def step(kc):
                spsum = scpool.tile([128, 512], F32, tag="sc")
                nc.tensor.matmul(
                    spsum,
                    lhsT=_r(ksel[r0:r0 + 64, kc * 128:(kc + 1) * 128]),
                    rhs=_r(qT[dt][r0:r0 + 64, csl]),
                    start=True, stop=True)
                esb = epool.tile([128, 512], F16, tag="esb")
                nc.scalar.activation(esb, spsum, EXP)
                pvo = (kc // 4) * VBLK + (kc % 4) * VSTRIDE
                nc.tensor.matmul(
                    apsum,
                    lhsT=_r(vsel[:, pvo:pvo + VSTRIDE]),
                    rhs=_r(esb),
                    start=(kc == 0), stop=(kc == 11))

            def finish():
                recip = rpool.tile([1, 512], F16, tag="recip")
                with nc.allow_low_precision(reason="f32r container is fp32"):
                    nc.vector.reciprocal(recip, apsum[64:65, :])
                rbp = scpool.tile([128, 512], F32, tag="sc")
                nc.tensor.matmul(rbp[0:64, :], lhsT=_r(ones_sb),
                                 rhs=_r(recip), start=True, stop=True)
                rb = epool.tile([128, 512], F16, tag="esb")
                nc.vector.tensor_copy(rb[0:64, :], rbp[0:64, :])
                nc.vector.tensor_mul(aT[dt][r0:r0 + 64, :],
                                     apsum[0:64, :], rb[0:64, :])

            return step, finish

        kv_next = [emit_gathers(0, h) for h in range(NH)]
        for c in range(NB):
            aT = [apool.tile([128, 512], F16, tag="attnT", name=f"aT{c}_{i}") for i in range(2)]
            kv = kv_next
            sf = [attn_head_ctx(c, h, kv[h][0], kv[h][1]) for h in range(NH)]
            for kc in range(12):
                for h in range(NH):
                    sf[h][0](kc)
                if kc == 1 and c + 1 < NB:
                    kv_next = [emit_gathers(c + 1, h) for h in range(NH)]
            for h in range(NH):
                sf[h][1]()

            # o-projection for this chunk
            for sub in range(4):
                oout = opool.tile([128, 1024], F32, tag="oo")
                for n in range(2):
                    opsum = oppool.tile([128, 512], F32, tag="sc")
                    nc.tensor.matmul(
                        opsum, lhsT=_r(aT[0][:, sub * 128:(sub + 1) * 128]),
                        rhs=_r(wo_sb[:, 0, n * 512:(n + 1) * 512]),
                        start=True, stop=False)
                    nc.tensor.matmul(
                        opsum, lhsT=_r(aT[1][:, sub * 128:(sub + 1) * 128]),
                        rhs=_r(wo_sb[:, 1, n * 512:(n + 1) * 512]),
                        start=False, stop=True)
                    nc.vector.tensor_copy(oout[:, n * 512:(n + 1) * 512], opsum)
                row = c * 512 + sub * 128
                nc.sync.dma_start(out[row:row + 128, :], oout)


def build_program():
    nc = bacc.Bacc()
    t = {
        "xT": nc.dram_tensor("xT", [DIN, S], F16, kind="ExternalInput"),
        "wq": nc.dram_tensor("wq", [DIN, 256], F16, kind="ExternalInput"),
        "wk": nc.dram_tensor("wk", [DIN, 256], F16, kind="ExternalInput"),
        "wqs": nc.dram_tensor("wqs", [DIN, 256], F16, kind="ExternalInput"),
        "wks": nc.dram_tensor("wks", [DIN, 256], F16, kind="ExternalInput"),
        "wv": nc.dram_tensor("wv", [DIN, 256], F16, kind="ExternalInput"),
        "wo": nc.dram_tensor("wo", [256, DIN], F16, kind="ExternalInput"),
        "cc": nc.dram_tensor("cc", [128, S], F32, kind="ExternalInput"),
        "ss": nc.dram_tensor("ss", [128, S], F32, kind="ExternalInput"),
        "mask": nc.dram_tensor("mask", [8, 8], F32, kind="ExternalInput"),
    }
    t["out"] = nc.dram_tensor("out", [S, DIN], F32, kind="ExternalOutput")
    from contextlib import ExitStack
    with TileContext(nc) as tc, ExitStack() as ctx:
        _build_body(nc, tc, t, ctx)
    nc.finalize()
    return nc


def make_core_inputs(x, wq, wk, wv, wo, cos, sin):
    """Host-side sharding/layout prep. Returns list of 8 per-core dicts."""
    x = np.asarray(x, np.float32)
    wq = np.asarray(wq, np.float32)
    wk = np.asarray(wk, np.float32)
    wv = np.asarray(wv, np.float32)
    wo = np.asarray(wo, np.float32)
    cos = np.asarray(cos, np.float32)
    sin = np.asarray(sin, np.float32)

    cosT, sinT = cos.T, sin.T                      # [32, S]
    cc = np.ascontiguousarray(np.tile(cosT, (4, 1)))
    ss = np.ascontiguousarray(np.tile(np.concatenate([-sinT, sinT], 0), (2, 1)))
    mask = np.zeros((8, 8), np.float32)
    for c in range(NB):
        for j in range(NB):
            if j > c:
                mask[c, j] = -(1.0e6 + 1000.0 * j)

    scale = 1.0 / np.sqrt(np.float32(HD))
    in_maps = []
    for core in range(8):
        b, hg = core // 4, core % 4
        colsl = slice(hg * 256, (hg + 1) * 256)
        def colswap(w):
            v = w.reshape(1024, 4, 2, 32)
            return np.ascontiguousarray(v[:, :, ::-1, :].reshape(1024, 256))

        wq_s = np.ascontiguousarray(wq[:, colsl] * scale)
        wk_s = np.ascontiguousarray(wk[:, colsl])
        in_maps.append({
            "xT": np.ascontiguousarray(x[b].T).astype(np.float16),
            "wq": wq_s.astype(np.float16),
            "wk": wk_s.astype(np.float16),
            "wqs": colswap(wq_s).astype(np.float16),
            "wks": colswap(wk_s).astype(np.float16),
            "wv": np.ascontiguousarray(wv[:, colsl]).astype(np.float16),
            "wo": np.ascontiguousarray(wo[hg * 256:(hg + 1) * 256, :]).astype(np.float16),
            "cc": cc, "ss": ss, "mask": mask,
        })
    return in_maps


_CACHE = {}


def _get_program():
    if "nc" not in _CACHE:
        _CACHE["nc"] = build_program()
    return _CACHE["nc"]


def run_cores(in_maps, **kwargs):
    nc = _get_program()
    return bass_utils.run_bass_kernel_spmd(nc, in_maps, list(range(8)), **kwargs)


def kernel(x, wq, wk, wv, wo, cos, sin):
    in_maps = make_core_inputs(x, wq, wk, wv, wo, cos, sin)
    res = run_cores(in_maps)
    outs = [res.results[i]["out"] for i in range(8)]
    full = np.empty((2, S, DIN), np.float32)
    for b in range(2):
        full[b] = outs[b * 4 + 0] + outs[b * 4 + 1] + outs[b * 4 + 2] + outs[b * 4 + 3]
    return full


# revision 25
# speedup vs baseline: 1.2367x; 1.2367x over previous
"""MoBA (Mixture of Block Attention) forward — Trainium2 Bass kernel.

Problem shapes: B=2, S=4096, DIM=1024, H=16 heads, HD=64, BLK=512, TOPK=3.

Sharding: 8 cores = 2 batches x 4 head-groups (4 heads each).
Each core computes, for its (batch b, heads hg*4..hg*4+3):
  q/k/v projections (head-sliced weights), RoPE, block gating + top-3,
  gathered block attention, and a partial output projection
  (row-slice of wo).  The host sums the 4 partials per batch.

Per-core layouts (all fp32):
  xT   [1024, 4096]  x[b] transposed (host-prepped)
  wq   [1024, 256]   head-sliced, pre-scaled by 1/sqrt(HD)
  wk   [1024, 256]
  wv   [1024, 256]
  wo   [256, 1024]   row-sliced
  cc   [128, 4096]   cos.T tiled 4x vertically
  ss   [128, 4096]   [-sin.T; +sin.T] tiled 2x
  mask [8, 8]        (c, blk): 0 if blk<=c else -(1e6+1000*blk)
  out  [4096, 1024]  partial output

On-device dataflow (per core):
  Phase A: for each s-tile of 512: Q^T,K^T ([dout,s] layout, 2 tiles of
    128 rows = 2 heads each) + RoPE (partition-swap via DMA) + per-block
    K sums; V in natural layout packed per (head, s-chunk-of-128) with an
    appended ones column (for softmax sums via the PV matmul).
  Phase C: per chunk c: gating scores at the chunk-mid query against
    block K-sums (tiny matmuls), +mask, top-3 via DVE max8/max_index;
    per head: gather the 3 selected K^T/V blocks via dynamic-offset
    SBUF->SBUF DMA, then 12x [QK^T -> exp -> PV] accumulating into a
    [65, 512] PSUM whose last row is sum(exp); normalize via reciprocal
    + K=1 broadcast matmul; o-projection of the finished chunk.
"""

import numpy as np

import concourse.bass as bass
import concourse.bacc as bacc
import concourse.mybir as mybir
from concourse.tile import TileContext
from concourse import bass_utils

F32 = mybir.dt.float32
F32R = mybir.dt.float32r
F16 = mybir.dt.float16
U32 = mybir.dt.uint32
EXP = mybir.ActivationFunctionType.Exp
COPY = mybir.ActivationFunctionType.Copy
AX_X = mybir.AxisListType.X
SP = mybir.EngineType.SP

S = 4096
DIN = 1024
HD = 64
NH = 4          # heads per core
BLK = 512
NB = 8          # blocks
NST = 8         # s-tiles of 512
TOPK = 3
VSTRIDE = 65    # V chunk cols (64 + ones col)
VBLK = 4 * VSTRIDE      # 260: one block of V per head
VHEAD = NB * VBLK       # 2080: all blocks of V per head


def _r(ap):
    return ap


def _build_body(nc, tc, t, ctx):
    ds = bass.ds
    xT, wq, wk, wv, wo, cc, ss, mask, out = (
        t["xT"], t["wq"], t["wk"], t["wv"], t["wo"], t["cc"], t["ss"],
        t["mask"], t["out"])
    wqs, wks = t["wqs"], t["wks"]

    res = ctx.enter_context(tc.tile_pool(name="resident", bufs=1))

    qT = [res.tile([128, S], F16, tag=f"qT{dt}", name=f"qT{dt}") for dt in range(2)]
    kT = [res.tile([128, S], F16, tag=f"kT{dt}", name=f"kT{dt}") for dt in range(2)]
    vsb = res.tile([128, NH * VHEAD], F16, tag="vsb")
    wo_sb = res.tile([128, 2, 1024], F16, tag="wo")
    bsum = [res.tile([128, NB], F32, tag=f"bsum{dt}", name=f"bsum{dt}") for dt in range(2)]
    qmid = [res.tile([128, NB], F32, tag=f"qmid{dt}", name=f"qmid{dt}") for dt in range(2)]
    mask_sb = res.tile([8, 8], F32, tag="mask")
    ones_sb = res.tile([1, 64], F16, tag="ones")

    nc.sync.dma_start(wo_sb, wo.rearrange("(c p) d -> p c d", p=128))
    nc.sync.dma_start(mask_sb, mask[:, :])
    # memset can't write f32r; memset fp32 scratch then DVE-copy (rounds)
    ones_f32 = res.tile([128, 128], F32, tag="onesf32")
    nc.vector.memset(ones_f32, 1.0)
    nc.vector.tensor_copy(ones_sb, ones_f32[0:1, 0:64])
    # ones columns of vsb: free offsets h*VHEAD + sub*VSTRIDE + 64
    vones = vsb[:].rearrange("p (h s e) -> p h s e", h=NH, s=NB * 4)[:, :, :, 64:65]
    nc.vector.tensor_copy(
        vones, ones_f32[:].rearrange("p (a b c) -> p a b c", a=NH, b=NB * 4))

    gidx = [res.tile([8, 8], U32, tag=f"gidx{h}", name=f"gidx{h}")
            for h in range(NH)]
    koff = [res.tile([8, 8], U32, tag=f"koff{h}", name=f"koff{h}")
            for h in range(NH)]
    voff = [res.tile([8, 8], U32, tag=f"voff{h}", name=f"voff{h}")
            for h in range(NH)]
    koff_w = [None] * NH
    voff_w = [None] * NH

    # ---------------- Phase A: projections + RoPE + block sums ----------
    with (
        tc.tile_pool(name="wqkv", bufs=1) as wpool,
        tc.tile_pool(name="xt", bufs=16) as xpool,
        tc.tile_pool(name="ccss", bufs=4) as cpool,
        tc.tile_pool(name="scr", bufs=10) as spool,
        tc.tile_pool(name="prjp", bufs=6, space="PSUM") as ppool,
    ):
        wq_sb = wpool.tile([128, 8, 256], F16, tag="wq")
        wk_sb = wpool.tile([128, 8, 256], F16, tag="wk")
        wqs_sb = wpool.tile([128, 8, 256], F16, tag="wqs")
        wks_sb = wpool.tile([128, 8, 256], F16, tag="wks")
        wv_sb = wpool.tile([128, 8, 256], F16, tag="wv")
        nc.sync.dma_start(wq_sb, wq.rearrange("(c p) d -> p c d", p=128))
        nc.sync.dma_start(wk_sb, wk.rearrange("(c p) d -> p c d", p=128))
        nc.sync.dma_start(wqs_sb, wqs.rearrange("(c p) d -> p c d", p=128))
        nc.sync.dma_start(wks_sb, wks.rearrange("(c p) d -> p c d", p=128))
        nc.sync.dma_start(wv_sb, wv.rearrange("(c p) d -> p c d", p=128))

        for st in range(NST):
            sl = slice(st * 512, (st + 1) * 512)
            xt = []
            for di in range(8):
                x_tile = xpool.tile([128, 512], F16, tag="xt")
                nc.sync.dma_start(x_tile, xT[di * 128:(di + 1) * 128, sl])
                xt.append(x_tile)
            cc_sl = cpool.tile([128, 512], F32, tag="ccss")
            ss_sl = cpool.tile([128, 512], F32, tag="ccss")
            nc.sync.dma_start(cc_sl, cc[:, sl])
            nc.sync.dma_start(ss_sl, ss[:, sl])

            for w_sb, w_swp, dst, is_k in ((wq_sb, wqs_sb, qT, False),
                                           (wk_sb, wks_sb, kT, True)):
                for dt in range(2):
                    psum = ppool.tile([128, 512], F32, tag="prj")
                    psws = ppool.tile([128, 512], F32, tag="prj")
                    for di in range(8):
                        nc.tensor.matmul(
                            psum,
                            lhsT=_r(w_sb[:, di, dt * 128:(dt + 1) * 128]),
                            rhs=_r(xt[di]),
                            start=(di == 0), stop=(di == 7))
                    for di in range(8):
                        nc.tensor.matmul(
                            psws,
                            lhsT=_r(w_swp[:, di, dt * 128:(dt + 1) * 128]),
                            rhs=_r(xt[di]),
                            start=(di == 0), stop=(di == 7))
                    # RoPE (swap pre-folded into w_swp): out = p*cc + ps*ss
                    t1 = spool.tile([128, 512], F32, tag="scr")
                    t2 = spool.tile([128, 512], F32, tag="scr")
                    full = spool.tile([128, 512], F32, tag="scr")
                    nc.vector.tensor_mul(t1, psum, cc_sl)
                    nc.vector.tensor_mul(t2, psws, ss_sl)
                    nc.vector.tensor_add(full, t1, t2)
                    nc.vector.tensor_copy(dst[dt][:, sl], full)
                    if is_k:
                        nc.vector.reduce_sum(bsum[dt][:, st:st + 1],
                                             full, axis=AX_X)
                    else:
                        nc.vector.tensor_copy(qmid[dt][:, st:st + 1],
                                              full[:, 256:257])

            for sub in range(4):
                vpsum = ppool.tile([128, 256], F32, tag="prj")
                for di in range(8):
                    nc.tensor.matmul(
                        vpsum,
                        lhsT=_r(xt[di][:, sub * 128:(sub + 1) * 128]),
                        rhs=_r(wv_sb[:, di, :]),
                        start=(di == 0), stop=(di == 7))
                gsub = st * 4 + sub
                vdst = vsb[:].rearrange(
                    "p (h s e) -> p h s e", h=NH, s=NB * 4)[:, :, gsub, 0:64]
                vsrc = vpsum[:].rearrange("p (h d) -> p h d", h=NH)
                nc.vector.tensor_copy(vdst, vsrc)

        # ------------ gating + top-3 (uses prjp psum pool) --------------
        for h in range(NH):
            dt, r0 = h // 2, (h % 2) * 64
            gpsum = ppool.tile([8, 8], F32, tag="prj", name=f"gp{h}")
            nc.tensor.matmul(gpsum, lhsT=qmid[dt][r0:r0 + 64, :],
                             rhs=bsum[dt][r0:r0 + 64, :],
                             start=True, stop=True)
            gat = res.tile([8, 8], F32, tag=f"gat{h}", name=f"gat{h}")
            nc.vector.tensor_add(gat, gpsum, mask_sb)
            gmax = res.tile([8, 8], F32, tag=f"gmax{h}", name=f"gmax{h}")
            nc.vector.max(gmax, gat)
            nc.vector.max_index(gidx[h], gmax, gat)
            koff_w[h] = nc.vector.tensor_scalar(koff[h], gidx[h], 512, None,
                                    op0=mybir.AluOpType.mult)
            voff_w[h] = nc.vector.tensor_scalar(voff[h], gidx[h], VBLK, h * VHEAD,
                                    op0=mybir.AluOpType.mult,
                                    op1=mybir.AluOpType.add)

    from concourse.tile_rust import add_dep_helper

    # ---------------- Phase C: attention + o-proj -----------------------
    prev_gathers = []
    with (
        tc.tile_pool(name="sel", bufs=9) as selpool,
        tc.tile_pool(name="esb", bufs=8) as epool,
        tc.tile_pool(name="att", bufs=4) as apool,
        tc.tile_pool(name="oo", bufs=2) as opool,
        tc.tile_pool(name="rcp", bufs=2) as rpool,
        tc.tile_pool(name="scp", bufs=4, space="PSUM") as scpool,
        tc.tile_pool(name="avp", bufs=4, space="PSUM") as avpool,
    ):
        oppool = scpool
        def emit_gathers(c, h):
            kld, ksels = nc.values_load_multi_w_load_instructions(
                koff[h][c:c + 1, 0:TOPK], engines=(SP,),
                min_val=0, max_val=7 * 512, skip_runtime_bounds_check=True)
            vld, vsels = nc.values_load_multi_w_load_instructions(
                voff[h][c:c + 1, 0:TOPK], engines=(SP,),
                min_val=h * VHEAD, max_val=h * VHEAD + 7 * VBLK,
                skip_runtime_bounds_check=True)
            for ld in kld:
                add_dep_helper(ld.ins, koff_w[h].ins, sync=True,
                               reason="reg_load RAW on koff")
            for ld in vld:
                add_dep_helper(ld.ins, voff_w[h].ins, sync=True,
                               reason="reg_load RAW on voff")
            for ld in (*kld, *vld):
                for g in prev_gathers:
                    add_dep_helper(ld.ins, g.ins, sync=False,
                                   reason="reg reuse WAR")
            ksel = selpool.tile([128, TOPK * 512], F16, tag="ksel",
                                name=f"ksel{c}_{h}")
            vsel = selpool.tile([128, TOPK * VBLK], F16, tag="vsel",
                                name=f"vsel{c}_{h}")
            prev_gathers.clear()
            dt = h // 2
            for sl_i in range(TOPK):
                prev_gathers.append(nc.sync.dma_start(
                    ksel[:, sl_i * 512:(sl_i + 1) * 512],
                    kT[dt][:, ds(ksels[sl_i], 512)]))
                prev_gathers.append(nc.sync.dma_start(
                    vsel[:, sl_i * VBLK:(sl_i + 1) * VBLK],
                    vsb[:, ds(vsels[sl_i], VBLK)]))
            return ksel, vsel

        def attn_head_ctx(c, h, ksel, vsel):
            dt, r0 = h // 2, (h % 2) * 64
            csl = slice(c * 512, (c + 1) * 512)
            apsum = avpool.tile([65, 512], F32, tag="av", name=f"ap{c}_{h}")

            def step(kc):
                spsum = scpool.tile([128, 512], F32, tag="sc")
                nc.tensor.matmul(
                    spsum,
                    lhsT=_r(ksel[r0:r0 + 64, kc * 128:(kc + 1) * 128]),
                    rhs=_r(qT[dt][r0:r0 + 64, csl]),
                    start=True, stop=True)
                esb = epool.tile([128, 512], F16, tag="esb")
                nc.scalar.activation(esb, spsum, EXP)
                pvo = (kc // 4) * VBLK + (kc % 4) * VSTRIDE
                nc.tensor.matmul(
                    apsum,
                    lhsT=_r(vsel[:, pvo:pvo + VSTRIDE]),
                    rhs=_r(esb),
                    start=(kc == 0), stop=(kc == 11))

            def finish():
                recip = rpool.tile([1, 512], F16, tag="recip")
                with nc.allow_low_precision(reason="f32r container is fp32"):
                    nc.vector.reciprocal(recip, apsum[64:65, :])
                rbp = scpool.tile([128, 512], F32, tag="sc")
                nc.tensor.matmul(rbp[0:64, :], lhsT=_r(ones_sb),
                                 rhs=_r(recip), start=True, stop=True)
                rb = epool.tile([128, 512], F16, tag="esb")
                nc.vector.tensor_copy(rb[0:64, :], rbp[0:64, :])
                nc.vector.tensor_mul(aT[dt][r0:r0 + 64, :],
                                     apsum[0:64, :], rb[0:64, :])

            return step, finish

        def make_oproj(c, aT):
            def emit(sub):
                oout = opool.tile([128, 1024], F32, tag="oo")
                for n in range(2):
                    opsum = oppool.tile([128, 512], F32, tag="sc")
                    nc.tensor.matmul(
                        opsum, lhsT=_r(aT[0][:, sub * 128:(sub + 1) * 128]),
                        rhs=_r(wo_sb[:, 0, n * 512:(n + 1) * 512]),
                        start=True, stop=False)
                    nc.tensor.matmul(
                        opsum, lhsT=_r(aT[1][:, sub * 128:(sub + 1) * 128]),
                        rhs=_r(wo_sb[:, 1, n * 512:(n + 1) * 512]),
                        start=False, stop=True)
                    nc.vector.tensor_copy(oout[:, n * 512:(n + 1) * 512], opsum)
                row = c * 512 + sub * 128
                nc.sync.dma_start(out[row:row + 128, :], oout)
            return emit

        kv_next = [emit_gathers(0, h) for h in range(NH)]
        prev_oproj = None
        for c in range(NB):
            aT = [apool.tile([128, 512], F16, tag="attnT", name=f"aT{c}_{i}") for i in range(2)]
            kv = kv_next
            sf = [attn_head_ctx(c, h, kv[h][0], kv[h][1]) for h in range(NH)]
            for kc in range(12):
                for h in range(NH):
                    sf[h][0](kc)
                if kc == 1 and c + 1 < NB:
                    kv_next = [emit_gathers(c + 1, h) for h in range(NH)]
                # drain previous chunk's o-projection one sub-tile per round
                if prev_oproj is not None and 2 <= kc < 6:
                    prev_oproj(kc - 2)
            for h in range(NH):
                sf[h][1]()
            prev_oproj = make_oproj(c, aT)
        for sub in range(4):
            prev_oproj(sub)


def build_program():
    nc = bacc.Bacc()
    t = {
        "xT": nc.dram_tensor("xT", [DIN, S], F16, kind="ExternalInput"),
        "wq": nc.dram_tensor("wq", [DIN, 256], F16, kind="ExternalInput"),
        "wk": nc.dram_tensor("wk", [DIN, 256], F16, kind="ExternalInput"),
        "wqs": nc.dram_tensor("wqs", [DIN, 256], F16, kind="ExternalInput"),
        "wks": nc.dram_tensor("wks", [DIN, 256], F16, kind="ExternalInput"),
        "wv": nc.dram_tensor("wv", [DIN, 256], F16, kind="ExternalInput"),
        "wo": nc.dram_tensor("wo", [256, DIN], F16, kind="ExternalInput"),
        "cc": nc.dram_tensor("cc", [128, S], F32, kind="ExternalInput"),
        "ss": nc.dram_tensor("ss", [128, S], F32, kind="ExternalInput"),
        "mask": nc.dram_tensor("mask", [8, 8], F32, kind="ExternalInput"),
    }
    t["out"] = nc.dram_tensor("out", [S, DIN], F32, kind="ExternalOutput")
    from contextlib import ExitStack
    with TileContext(nc) as tc, ExitStack() as ctx:
        _build_body(nc, tc, t, ctx)
    nc.finalize()
    return nc


def make_core_inputs(x, wq, wk, wv, wo, cos, sin):
    """Host-side sharding/layout prep. Returns list of 8 per-core dicts."""
    x = np.asarray(x, np.float32)
    wq = np.asarray(wq, np.float32)
    wk = np.asarray(wk, np.float32)
    wv = np.asarray(wv, np.float32)
    wo = np.asarray(wo, np.float32)
    cos = np.asarray(cos, np.float32)
    sin = np.asarray(sin, np.float32)

    cosT, sinT = cos.T, sin.T                      # [32, S]
    cc = np.ascontiguousarray(np.tile(cosT, (4, 1)))
    ss = np.ascontiguousarray(np.tile(np.concatenate([-sinT, sinT], 0), (2, 1)))
    mask = np.zeros((8, 8), np.float32)
    for c in range(NB):
        for j in range(NB):
            if j > c:
                mask[c, j] = -(1.0e6 + 1000.0 * j)

    scale = 1.0 / np.sqrt(np.float32(HD))
    in_maps = []
    for core in range(8):
        b, hg = core // 4, core % 4
        colsl = slice(hg * 256, (hg + 1) * 256)
        def colswap(w):
            v = w.reshape(1024, 4, 2, 32)
            return np.ascontiguousarray(v[:, :, ::-1, :].reshape(1024, 256))

        wq_s = np.ascontiguousarray(wq[:, colsl] * scale)
        wk_s = np.ascontiguousarray(wk[:, colsl])
        in_maps.append({
            "xT": np.ascontiguousarray(x[b].T).astype(np.float16),
            "wq": wq_s.astype(np.float16),
            "wk": wk_s.astype(np.float16),
            "wqs": colswap(wq_s).astype(np.float16),
            "wks": colswap(wk_s).astype(np.float16),
            "wv": np.ascontiguousarray(wv[:, colsl]).astype(np.float16),
            "wo": np.ascontiguousarray(wo[hg * 256:(hg + 1) * 256, :]).astype(np.float16),
            "cc": cc, "ss": ss, "mask": mask,
        })
    return in_maps


_CACHE = {}


def _get_program():
    if "nc" not in _CACHE:
        _CACHE["nc"] = build_program()
    return _CACHE["nc"]


def run_cores(in_maps, **kwargs):
    nc = _get_program()
    return bass_utils.run_bass_kernel_spmd(nc, in_maps, list(range(8)), **kwargs)


def kernel(x, wq, wk, wv, wo, cos, sin):
    in_maps = make_core_inputs(x, wq, wk, wv, wo, cos, sin)
    res = run_cores(in_maps)
    outs = [res.results[i]["out"] for i in range(8)]
    full = np.empty((2, S, DIN), np.float32)
    for b in range(2):
        full[b] = outs[b * 4 + 0] + outs[b * 4 + 1] + outs[b * 4 + 2] + outs[b * 4 + 3]
    return full
